# revision 14
# baseline (speedup 1.0000x reference)
"""CRF NLL loss kernel for 8 Trainium2 NeuronCores (Bass/Tile) — v3.

Data-parallel, 32 sequences per core.  Forward algorithm in the exp domain:
E_t = g_t * (A @ E_{t-1}) with A = exp(T) bf16 and g_t = exp(feat_t - C0).

v3: contiguous per-(b, t-half) DMAs with 2KB packets (partition = t//4,
4 t per partition row) — scattered per-segment loads thrashed HBM rows.
S=30 chains (W=3 burn-in, DEL=34) cut serial rounds to 37, run as FOUR
independent sub-chains (two per t-half store) so the post-stream tail is
two interleaved chains instead of one wide serial one.  PE transposes
(bf16) + ACT-exp-from-PSUM build the transposed g stores; segment-0 init
is copied from the t=0 column of store 0.  Gold: one-hot diff on GPSIMD,
fused (d==0)*feat accumulate on DVE.  Host does the O(B) combine + the
tags/T transition term.
"""

import numpy as np

B, L, C = 256, 1024, 128
NCORES = 8
BC = B // NCORES
C0 = 5.3
S = 30
W = 3
DEL = (L - 1 - W) // S    # 34
NR = W + DEL              # 37
assert S * DEL + W == L - 1

# sub-chains: (first seg, n segs, store th)
CHAINS = [(0, 8, 0), (8, 7, 0), (15, 8, 1), (23, 7, 1)]
ECOLS = [ns * BC for (_, ns, _) in CHAINS]      # 256, 224, 256, 224
EOFF = [0, 256, 480, 736]                        # col offsets in ef/cw
ETOT = 960

_cache = {}


def _build():
    import concourse.bacc as bacc
    import concourse.mybir as mybir
    from concourse.tile import TileContext

    f32 = mybir.dt.float32
    bf16 = mybir.dt.bfloat16
    i32 = mybir.dt.int32
    MUL = mybir.AluOpType.mult
    SUB = mybir.AluOpType.subtract
    EQ = mybir.AluOpType.is_equal
    ADD = mybir.AluOpType.add
    EXP = mybir.ActivationFunctionType.Exp
    COPY = mybir.ActivationFunctionType.Copy

    nc = bacc.Bacc("TRN2")
    feats = nc.dram_tensor("feats", [BC, L, C], f32, kind="ExternalInput")
    tags = nc.dram_tensor("tags", [BC, L], i32, kind="ExternalInput")
    Tm = nc.dram_tensor("T", [C, C], f32, kind="ExternalInput")
    ef_o = nc.dram_tensor("ef", [C, ETOT], f32, kind="ExternalOutput")
    cw_o = nc.dram_tensor("cw", [1, ETOT], f32, kind="ExternalOutput")
    gold_o = nc.dram_tensor("gold", [C, 64], f32, kind="ExternalOutput")

    with TileContext(nc) as tc:
        with (
            tc.tile_pool(name="const", bufs=1) as cp,
            tc.tile_pool(name="gstore", bufs=1) as gp,
            tc.tile_pool(name="sin", bufs=4) as sip,
            tc.tile_pool(name="dtile", bufs=2) as dtp,
            tc.tile_pool(name="state", bufs=1) as st,
            tc.tile_pool(name="small", bufs=2) as sm,
            tc.tile_pool(name="ptr", bufs=2, space="PSUM") as ptp,
            tc.tile_pool(name="pmm", bufs=1, space="PSUM") as pmp,
            tc.tile_pool(name="paux", bufs=1, space="PSUM") as pxp,
        ):
            # ---- constants
            t_f32 = cp.tile([C, C], f32, tag="t_f32")
            nc.sync.dma_start(t_f32[:], Tm[:])
            AB = cp.tile([C, C], bf16, tag="AB")
            nc.scalar.activation(AB[:], t_f32[:], EXP)
            AF = cp.tile([C, C], bf16, tag="AF")
            nc.sync.dma_start_transpose(AF[:], AB[:])
            ones_col = cp.tile([C, 1], bf16, tag="ones_col")
            nc.vector.memset(ones_col[:], 1.0)
            biasc = cp.tile([C, 1], f32, tag="biasc")
            nc.vector.memset(biasc[:], -C0)

            iota_i = cp.tile([C, C], i32, tag="iota_i")
            nc.gpsimd.iota(iota_i[:], pattern=[[1, C]], base=0,
                           channel_multiplier=0)
            iota_row = cp.tile([C, C], bf16, tag="iota_row")
            nc.vector.tensor_copy(iota_row[:], iota_i[:])
            iota_pi = cp.tile([C, 1], i32, tag="iota_pi")
            nc.gpsimd.iota(iota_pi[:], pattern=[[0, 1]], base=0,
                           channel_multiplier=1)
            iota_pb = cp.tile([C, 1], bf16, tag="iota_pb")
            nc.vector.tensor_copy(iota_pb[:], iota_pi[:])
            ident = cp.tile([C, C], bf16, tag="ident")
            nc.vector.tensor_tensor(out=ident[:], in0=iota_row[:],
                                    in1=iota_pb[:].broadcast_to([C, C]),
                                    op=EQ)
            ident_f = cp.tile([C, C], f32, tag="ident_f")
            nc.vector.tensor_copy(ident_f[:], ident[:])

            tg = cp.tile([BC, L], i32, tag="tg")
            nc.sync.dma_start(tg[:], tags[:])
            tg_bf = cp.tile([BC, L], bf16, tag="tg_bf")
            nc.vector.tensor_copy(tg_bf[:], tg[:])

            # tags re-striped: tgX[pt, th*128 + u*32 + b] = tags[b, t],
            # t = 512*th + 4*pt + u
            tgX = cp.tile([C, 256], bf16, tag="tgX")
            for th in range(2):
                psg = pxp.tile([C, 128], bf16, name="psg", tag="psg",
                               bufs=1)
                for u in range(4):
                    sl = tg_bf[:, 512 * th + u: 512 * th + u + 509: 4]
                    nc.tensor.transpose(psg[:, u * BC:(u + 1) * BC], sl,
                                        ident[0:BC, 0:BC])
                nc.vector.tensor_copy(tgX[:, th * 128:(th + 1) * 128],
                                      psg[:])

            emit = cp.tile([C, 64], f32, tag="emit")
            nc.vector.memset(emit[:], 0.0)

            # ---- g stores: gT[th] col = b*512 + (t - 512*th)
            gT = [gp.tile([C, 32 * 512], bf16, name=f"gT{k}", tag=f"gT{k}")
                  for k in range(2)]

            E = [[st.tile([C, ECOLS[ci]], bf16, name=f"E{ci}_{i}",
                          tag=f"E{ci}_{i}") for i in range(3)]
                 for ci in range(4)]

            junk = cp.tile([C, 512], bf16, tag="junk")
            cwsg = cp.tile([1, ETOT], f32, tag="cwsg")

            def stream_tile(b, th):
                s_in = sip.tile([C, 512], f32, name="s_in", tag="s_in")
                src = feats[b: b + 1, 512 * th: 512 * (th + 1), :] \
                    .rearrange("o (pt u) c -> (o pt) (u c)", u=4)
                nc.sync.dma_start(s_in[:], src)
                tp = ptp.tile([C, 512], f32, name="tp", tag="tp")
                for u in range(4):
                    nc.tensor.transpose(tp[:, u * 128:(u + 1) * 128],
                                        s_in[:, u * C:(u + 1) * C],
                                        ident_f[:])
                # gT col = x*32 + b = pt*128 + u*32 + b
                ov = gT[th][:] \
                    .rearrange("p (pt u b2) -> p u pt b2", u=4, b2=BC) \
                    [:, :, :, b: b + 1]
                iv = tp[:].rearrange("p (u pt) -> p u pt", pt=128)
                nc.scalar.activation(ov, iv, EXP, bias=biasc[:, 0:1])

                d = dtp.tile([C, 512], bf16, name="d", tag="d")
                nc.gpsimd.tensor_tensor(
                    out=d[:].rearrange("p (u c) -> p u c", u=4),
                    in0=iota_row[:].rearrange("p (o c) -> p o c", o=1)
                    .broadcast_to([C, 4, C]),
                    in1=tgX[:, th * 128 + b: th * 128 + b + 97: 32]
                    .rearrange("p (u o) -> p u o", o=1)
                    .broadcast_to([C, 4, C]),
                    op=SUB)
                nc.vector.scalar_tensor_tensor(
                    out=junk[:], in0=d[:], scalar=0.0, in1=s_in[:],
                    op0=EQ, op1=MUL,
                    accum_out=emit[:, th * 32 + b: th * 32 + b + 1])

            def estore_init():
                for ci in range(4):
                    nc.vector.memset(E[ci][0][:], 1.0)
                # chain 0 segment 0 starts from alpha_0 = g(t=0)
                nc.vector.tensor_copy(E[0][0][:, 0:BC], gT[0][:, 0:BC])

            def chain_round(ci, j):
                s0, ns, th = CHAINS[ci]
                ps = pmp.tile([C, ECOLS[ci]], f32, name=f"ps{ci}",
                              tag=f"ps{ci}")
                nc.tensor.matmul(ps[:], AF[:], E[ci][(j - 1) % 3][:],
                                 start=True, stop=True)
                ov = E[ci][j % 3][:].rearrange("p (s b) -> p s b", b=BC)
                iv = ps[:].rearrange("p (s b) -> p s b", b=BC)
                gv = gT[th][:].rearrange("p (x b) -> p x b", b=BC)
                x0 = 34 * s0 + j - 512 * th
                xe = x0 + (ns - 1) * 34
                if x0 >= 0 and xe < 512:
                    nc.vector.tensor_tensor(
                        out=ov, in0=iv,
                        in1=gv[:, x0: xe + 1: 34, :], op=MUL)
                elif x0 < 0:
                    # first segment's t sits in the previous store
                    gv0 = gT[0][:].rearrange("p (x b) -> p x b", b=BC)
                    nc.vector.tensor_tensor(
                        out=ov[:, 0:1, :], in0=iv[:, 0:1, :],
                        in1=gv0[:, 512 + x0: 513 + x0, :], op=MUL)
                    nc.vector.tensor_tensor(
                        out=ov[:, 1:ns, :], in0=iv[:, 1:ns, :],
                        in1=gv[:, x0 + 34: xe + 1: 34, :], op=MUL)
                else:
                    # last segment's t spills into the next store
                    gv1 = gT[1][:].rearrange("p (x b) -> p x b", b=BC)
                    nc.vector.tensor_tensor(
                        out=ov[:, 0:ns - 1, :], in0=iv[:, 0:ns - 1, :],
                        in1=gv[:, x0: xe - 34 + 1: 34, :], op=MUL)
                    nc.vector.tensor_tensor(
                        out=ov[:, ns - 1:ns, :], in0=iv[:, ns - 1:ns, :],
                        in1=gv1[:, xe - 512: xe - 511, :], op=MUL)
                if j == W:
                    cwp = pxp.tile([1, 256], f32, name="cwp",
                                   tag="cwp", bufs=1)
                    nc.tensor.matmul(cwp[:, 0:ECOLS[ci]], ones_col[:],
                                     E[ci][j % 3][:], start=True, stop=True)
                    nc.scalar.copy(cwsg[:, EOFF[ci]: EOFF[ci] + ECOLS[ci]],
                                   cwp[:, 0:ECOLS[ci]])

            # ---- schedule
            for b in range(BC):
                stream_tile(b, 0)
            estore_init()
            # chains 0/1 (store 0) run while store 1 streams; rounds 36,37
            # of chain 1 touch store 1 so they wait until after.
            for b in range(BC):
                stream_tile(b, 1)
                j = b + 1
                if j <= NR:
                    chain_round(0, j)
                if j <= 32:
                    chain_round(1, j)
            for j in range(33, 36):
                chain_round(1, j)
            for j in range(1, NR + 1):
                chain_round(2, j)
                chain_round(3, j)
                if j >= 36:
                    chain_round(1, j)

            nc.sync.dma_start(cw_o[:], cwsg[:])
            for ci in range(4):
                ef = sm.tile([C, ECOLS[ci]], f32, name=f"ef{ci}",
                             tag=f"ef{ci}")
                nc.scalar.activation(ef[:], E[ci][NR % 3][:], COPY)
                nc.sync.dma_start(
                    ef_o[:, EOFF[ci]: EOFF[ci] + ECOLS[ci]], ef[:])
            nc.sync.dma_start(gold_o[:], emit[:])

    nc.compile()
    return nc


def _get_nc():
    if "nc" not in _cache:
        _cache["nc"] = _build()
    return _cache["nc"]


def kernel(feats, tags, T, _trace=False, _trace_kwargs=None):
    from concourse.bass_utils import run_bass_kernel_spmd

    feats = np.ascontiguousarray(feats, dtype=np.float32)
    tags = np.ascontiguousarray(tags, dtype=np.int32)
    T = np.ascontiguousarray(T, dtype=np.float32)

    nc = _get_nc()
    in_maps = []
    for c in range(NCORES):
        sl = slice(c * BC, (c + 1) * BC)
        in_maps.append({"feats": feats[sl], "tags": tags[sl], "T": T})
    res = run_bass_kernel_spmd(nc, in_maps, core_ids=list(range(NCORES)),
                               trace=_trace, **(_trace_kwargs or {}))
    if _trace:
        _cache["last_results"] = res

    logZ = np.zeros(B)
    gold_total = 0.0
    for c, r in enumerate(res.results):
        sl = slice(c * BC, (c + 1) * BC)
        ef = r["ef"].astype(np.float64)          # [C, 960]
        cw = r["cw"].astype(np.float64)[0]       # [960]
        lf = np.zeros(B)
        lzc = np.zeros(BC)
        for ci, (s0, ns, _) in enumerate(CHAINS):
            sl_c = slice(EOFF[ci], EOFF[ci] + ns * BC)
            lfc = np.log(ef[:, sl_c].sum(axis=0)).reshape(ns, BC)
            cwc = np.log(cw[sl_c]).reshape(ns, BC)
            corr = cwc.copy()
            if ci == 0:
                corr[0, :] = 0.0         # segment 0: no burn-in
            lzc += (lfc - corr).sum(axis=0)
        logZ[sl] = lzc + L * C0
        gold_total += float(r["gold"].astype(np.float64).sum())
    gold_total += float(T.astype(np.float64)[tags[:, 1:], tags[:, :-1]].sum())
    loss = logZ.mean() - gold_total / B
    return np.float32(loss)
